# revision 30
# baseline (speedup 1.0000x reference)
"""Trainium2 Bass kernel for nn_Attention_9345848836379 (two-stream attention).

Sharding: 8 cores = 2 batches x 4 head-groups (4 heads, head-group width 256).
Per core: QKV projection for its head-group (both streams), attention, and a
row-sharded c_proj partial output.  The host sums the 4 partials per batch at
gather time (the all-reduce of the sharding hint, done on unshard).

v2 design (bf16 pipeline):
  - All matmul operands bf16 (FWL fast weight loads, half DMA, 2x DVE modes);
    PSUM accumulation stays fp32.  Outputs are written bf16 and upcast on host.
  - Layouts all transposed (nothing needs an on-chip transpose):
      xT [C, T], q^T/k^T packed [128, 2, T] (head h at partition (h%2)*64,
      tile h//2), S^T [k, q] blocks, v_aug [T, 65/head] with a ones column so
      PV gives O^T rows = head dims plus row 64 = softmax denominator Z,
      y^T [256, T], and c_proj computed as o^T = Wp^T y^T so the bias is a
      per-partition ACT bias and the host transposes on gather.
  - Softmax 1/Z via ACT Ln then Exp(scale=-1) batched over [1, 4*512] rows
    (same natural_log_exp table set as the score exp; DVE reciprocal on a
    [1,512] row costs 4us, this costs ~2x2us per (stream, qt)).
  - Score exp batched over 2-PSUM-bank [128, 1024] tiles, windowed to skip
    the causally-masked left region; PV matmuls window identically so the
    skipped columns are never read.
  - Hat-stream merged softmax: strict-causal scores vs star keys, with the
    diagonal exp(qh.kh) injected as diag(e) [128,128] tiles added into the
    u-tiles (picks up v_s and Z via v_aug), plus tiny N=128 matmuls
    vh^T @ diag(e) for the v_h term.

Hard constraints honored (probed previously):
  - matmul operands at SBUF base partition 0/64 (lhsT and rhs must match);
    matmul output at PSUM partition 0; one accumulation group keeps a single
    tile_position.
Fast path hard-codes the structural masks (star causal, hat diagonal);
kernel() verifies and falls back to numpy for arbitrary masks.
"""

import math
from contextlib import ExitStack

import numpy as np

B, T, C, H = 2, 1024, 1024, 16
D = C // H                      # 64
G = 8                           # cores
HG = 4                          # head-groups
HPG = H // HG                   # 4 heads per group
W_G = HPG * D                   # 256 = head-group width
SCALE = 1.0 / math.sqrt(D)      # 0.125
NT = T // 512                   # 2 q-tiles of 512
KT = T // 128                   # 8 k-tiles of 128

_BUILD_CACHE = {}


def _build_fast():
    """Build the SPMD kernel (same program for all 8 cores)."""
    import concourse.bacc as bacc
    import concourse.tile as tile
    from concourse import mybir

    BF = mybir.dt.bfloat16
    F32 = mybir.dt.float32
    AF = mybir.ActivationFunctionType

    # Force Exp/Ln/Identity to resolve to the one table set that has all
    # three, else bacc alternates exp_and_others <-> natural_log loads
    # (measured: 9 ACT_TABLE_LOADs, ~1.3us each plus pipeline drains).
    # Set ids index act_info.json, so strip functions rather than filter.
    if not getattr(bacc, "_act_tables_pinned", False):
        _orig_get_tables = bacc.get_activation_tables

        def _pinned_tables(arch):
            tabs = _orig_get_tables(arch)
            pin = {AF.Exp, AF.Ln, AF.Identity, AF.Copy}
            for name, fns in tabs.items():
                if name != "natural_log_exp_and_others":
                    fns -= pin
            return tabs

        bacc.get_activation_tables = _pinned_tables
        bacc._act_tables_pinned = True

    nc = bacc.Bacc("TRN2", target_bir_lowering=False, debug=False)

    def dt_in(n, s, d=BF):
        return nc.dram_tensor(n, s, d, kind="ExternalInput").ap()

    xT_s = dt_in("xT_s", [C, T])
    xT_h = dt_in("xT_h", [C, T])
    wq = dt_in("wq", [C, W_G])
    wk = dt_in("wk", [C, W_G])
    wv = dt_in("wv", [C, W_G])
    wp = dt_in("wp", [W_G, C])
    bq_t = dt_in("bq_t", [128, 2], F32)      # head-pair bias columns
    bk_t = dt_in("bk_t", [128, 2], F32)
    bv_row = dt_in("bv_row", [1, W_G])
    bp_cols = dt_in("bp_cols", [128, 8], F32)
    ones_in = dt_in("ones_in", [128, 128])
    ident = dt_in("ident", [128, 128])
    diag_incl = dt_in("diag_incl", [128, 128])     # keep k<=q within diag block
    diag_strict = dt_in("diag_strict", [128, 128])  # keep k<q
    o_star = nc.dram_tensor("o_star", [C, T], BF, kind="ExternalOutput").ap()
    o_hat = nc.dram_tensor("o_hat", [C, T], BF, kind="ExternalOutput").ap()

    with tile.TileContext(nc) as tc, ExitStack() as ctx:
        pbig = ctx.enter_context(tc.tile_pool(name="pbig", bufs=2))
        pw = ctx.enter_context(tc.tile_pool(name="pw", bufs=4))
        pqk = ctx.enter_context(tc.tile_pool(name="pqk", bufs=4))
        pv = ctx.enter_context(tc.tile_pool(name="pv", bufs=2))
        pu = ctx.enter_context(tc.tile_pool(name="pu", bufs=12))
        poh = ctx.enter_context(tc.tile_pool(name="poh", bufs=2))
        pyt = ctx.enter_context(tc.tile_pool(name="pyt", bufs=2))
        pout = ctx.enter_context(tc.tile_pool(name="pout", bufs=4))
        pz = ctx.enter_context(tc.tile_pool(name="pz", bufs=2))
        ped = ctx.enter_context(tc.tile_pool(name="ped", bufs=10))
        pg = ctx.enter_context(tc.tile_pool(name="pg", bufs=2))
        pc1 = ctx.enter_context(tc.tile_pool(name="pc1", bufs=1))
        # PSUM: 8 banks = psS 4x1 + psO 2x1 + psC 2x1
        psS = ctx.enter_context(tc.tile_pool(name="psS", bufs=4, space="PSUM"))
        psO = ctx.enter_context(tc.tile_pool(name="psO", bufs=2, space="PSUM"))
        psC = ctx.enter_context(tc.tile_pool(name="psC", bufs=2, space="PSUM"))

        # ---- constants ------------------------------------------------
        onesb = pc1.tile([128, 128], BF)
        idn = pc1.tile([128, 128], BF)
        d_incl = pc1.tile([128, 128], BF)
        d_strict = pc1.tile([128, 128], BF)
        bq = pc1.tile([128, 2], F32)
        bk = pc1.tile([128, 2], F32)
        bvr = pc1.tile([1, W_G], BF)
        bpc = pc1.tile([128, 8], F32)
        const_dmas = [(onesb, ones_in), (idn, ident), (d_incl, diag_incl),
                      (d_strict, diag_strict), (bq, bq_t), (bk, bk_t),
                      (bvr, bv_row), (bpc, bp_cols)]

        # ---- stage inputs ---------------------------------------------
        sxT = {}
        xviews = {}
        for st, dram in (("s", xT_s), ("h", xT_h)):
            sxT[st] = pbig.tile([128, KT, T], BF, tag="big", name=f"sxT_{st}")
            xviews[st] = dram.rearrange("(ct p) t -> p ct t", p=128)
        sw_ = {name: pw.tile([128, KT, W_G], BF, tag="w", name=f"sw_{name}")
               for name in ("q", "k", "v")}
        swp = pw.tile([128, 2, C], BF, tag="w")
        wviews = {"q": wq.rearrange("(ct p) n -> p ct n", p=128),
                  "k": wk.rearrange("(ct p) n -> p ct n", p=128),
                  "v": wv.rearrange("(ct p) n -> p ct n", p=128)}
        dma_engs = [nc.sync, nc.scalar, nc.gpsimd]
        # constants first (tiny; the ones tile feeds the PE warm-up burst)
        for t, dram in const_dmas:
            nc.gpsimd.dma_start(t, dram)
        # critical path: x_star + w_q + w_k; w_v afterwards (v_wave runs
        # after q/k anyway and its DMA would steal critical-window bandwidth)
        for ct in range(KT):
            e0 = dma_engs[ct % 3]
            e1 = dma_engs[(ct + 1) % 3]
            e0.dma_start(sxT["s"][:, ct, :], xviews["s"][:, ct, :])
            e1.dma_start(sw_["q"][:, ct, :], wviews["q"][:, ct, :])
            e1.dma_start(sw_["k"][:, ct, :], wviews["k"][:, ct, :])
        for ct in range(KT):
            dma_engs[ct % 3].dma_start(sw_["v"][:, ct, :], wviews["v"][:, ct, :])

        # PE warm-up: dummy matmuls on the resident ones tile into an
        # otherwise-idle psO slot. Costs only DMA-wait idle time and flips
        # the HAM clock gate to 8/8 (~3.4us of sustained PE busy) before the
        # real QKV matmuls arrive, so they run at 2.4 GHz instead of 1.2.
        # Four disjoint output ranges so the matmuls don't WAW-chain through
        # per-instruction semaphores.
        warm = psO.tile([64, 512], F32, tag="po", name="warmup")
        for j in range(24):
            nc.tensor.matmul(warm[:, (j % 4) * 128:(j % 4) * 128 + 128],
                             onesb[:, 0:64], onesb, start=True, stop=True)

        # ---- QKV projections ------------------------------------------
        qkT = {}

        def qk_subwave(mat, wname, st, bias, mt):
            """Half of a q^T/k^T projection (one head pair): ct-major
            [128,512] psC chains with copy-outs applying the bias."""
            if mat not in qkT:
                qkT[mat] = pqk.tile([128, 2, T], BF, tag="qk", name=f"qk_{mat}")
            dst = qkT[mat]
            for nt in range(NT):
                a = psC.tile([128, 512], F32, tag="c", name=f"qkacc{mat}{mt}{nt}")
                for ct in range(KT):
                    nc.tensor.matmul(
                        a,
                        sw_[wname][:, ct, mt * 128:(mt + 1) * 128],
                        sxT[st][:, ct, nt * 512:(nt + 1) * 512],
                        start=(ct == 0), stop=(ct == KT - 1))
                if nt == 0:
                    nc.scalar.activation(dst[:, mt, nt * 512:(nt + 1) * 512], a,
                                         AF.Identity, bias=bias[:, mt:mt + 1],
                                         scale=1.0)
                else:
                    nc.vector.tensor_scalar_add(dst[:, mt, nt * 512:(nt + 1) * 512],
                                                a, bias[:, mt:mt + 1])

        def qk_wave(mat, wname, st, bias):
            for mt in range(2):
                qk_subwave(mat, wname, st, bias, mt)

        def qk_star_interleaved():
            """q_s (both head pairs, 2-bank psS chains) and k_s pair 0
            ([128,512] psC chains) together, ct-major, so the projection
            tracks the per-chunk DMA arrival instead of serializing wave by
            wave; k_s pair 1 follows as a dense subwave."""
            qkT["qs"] = pqk.tile([128, 2, T], BF, tag="qk", name="qk_qs")
            qkT["ks"] = pqk.tile([128, 2, T], BF, tag="qk", name="qk_ks")
            qacc = {(mt, nt): psS.tile([128, 512], F32, tag="s",
                                       name=f"qkaccqs{mt}{nt}")
                    for mt in range(2) for nt in range(NT)}
            kacc = [psC.tile([128, 512], F32, tag="c", name=f"qkaccks0{nt}")
                    for nt in range(NT)]
            for ct in range(KT):
                for mt in range(2):
                    for nt in range(NT):
                        nc.tensor.matmul(
                            qacc[mt, nt],
                            sw_["q"][:, ct, mt * 128:(mt + 1) * 128],
                            sxT["s"][:, ct, nt * 512:(nt + 1) * 512],
                            start=(ct == 0), stop=(ct == KT - 1))
                for nt in range(NT):
                    nc.tensor.matmul(
                        kacc[nt],
                        sw_["k"][:, ct, 0:128],
                        sxT["s"][:, ct, nt * 512:(nt + 1) * 512],
                        start=(ct == 0), stop=(ct == KT - 1))
            # copy-outs split across ACT and DVE: they all become ready at
            # once (every chain needs the last chunk) and attention S blocks
            # on them, so halve the serial chain with two engines
            for mt in range(2):
                for nt in range(NT):
                    dst = qkT["qs"][:, mt, nt * 512:(nt + 1) * 512]
                    src = qacc[mt, nt]
                    if nt == 0:
                        nc.scalar.activation(dst, src, AF.Identity,
                                             bias=bq[:, mt:mt + 1], scale=1.0)
                    else:
                        nc.vector.tensor_scalar_add(dst, src, bq[:, mt:mt + 1])
            for nt in range(NT):
                dst = qkT["ks"][:, 0, nt * 512:(nt + 1) * 512]
                if nt == 0:
                    nc.scalar.activation(dst, kacc[nt], AF.Identity,
                                         bias=bk[:, 0:1], scale=1.0)
                else:
                    nc.vector.tensor_scalar_add(dst, kacc[nt], bk[:, 0:1])
            qk_subwave("ks", "k", "s", bk, 1)

        def v_subwave(st, dst, aug, half):
            """v [T, 256] for 2 kt: two ki-chains share the psC bank (only
            the first issues start=True: start clears the whole bank's
            has_written bits; the second chain's first write then
            overwrites-and-sets on the cleared bits)."""
            acc = psC.tile([128, 512], F32, tag="c", name=f"vacc{st}{half}")
            for ct in range(KT):
                for ki in range(2):
                    kt = half * 2 + ki
                    nc.tensor.matmul(
                        acc[:, ki * 256:(ki + 1) * 256],
                        sxT[st][:, ct, kt * 128:(kt + 1) * 128],
                        sw_["v"][:, ct, :],
                        start=(ct == 0 and ki == 0), stop=False)
            for ki in range(2):
                nc.tensor.matmul(acc[:, ki * 256:(ki + 1) * 256],
                                 onesb[0:1, :], bvr, start=False, stop=True)
            src = acc.rearrange("p (k h c) -> p k h c", k=2, c=64)
            if aug:
                dv = dst[:, half * 2:half * 2 + 2, :].rearrange(
                    "p k (h c) -> p k h c", c=65)[:, :, :, 0:64]
            else:
                dv = dst[:, half * 2:half * 2 + 2, :].rearrange(
                    "p k (h c) -> p k h c", c=64)
            nc.vector.tensor_copy(dv, src)

        def v_wave(st, dst, aug):
            for half in range(4):
                v_subwave(st, dst, aug, half)

        vs_aug = pv.tile([128, KT, HPG * 65], BF, tag="v")
        vh_raw = pv.tile([128, KT, W_G], BF, tag="v")

        # ---- hat diag prep helpers ------------------------------------
        eT = pc1.tile([128, KT * HPG], F32)

        def e_rows():
            """eT[:, kt*4+h] = exp(diag(q_h k_h^T)/8) for k-partition layout."""
            ed = psC.tile([128, 2 * KT * HPG], F32, tag="c", name="ed")
            for h in range(HPG):
                hb, hp = (h % 2) * 64, h // 2
                gt = pg.tile([128, T], BF, tag="g", name=f"gt{h}")
                nc.vector.tensor_mul(gt[hb:hb + 64, :], qkT["qh"][hb:hb + 64, hp, :],
                                     qkT["kh"][hb:hb + 64, hp, :])
                for kt in range(KT):
                    j = kt * HPG + h
                    nc.tensor.matmul(ed[:, 2 * j:2 * j + 2],
                                     gt[hb:hb + 64, kt * 128:(kt + 1) * 128],
                                     onesb[hb:hb + 64, 0:2], start=True, stop=True)
            nc.scalar.activation(
                eT, ed.rearrange("p (j two) -> p j two", two=2)[:, :, 0:1],
                AF.Exp, scale=SCALE)

        # ---- attention ------------------------------------------------
        def attention_pair(stream, qt, pair):
            """One q-window of attention for one head pair (heads at base
            partitions 0/64 issued adjacently for row-group concurrency).
            S^T blocks windowed to the causal region; exp batched per 2-bank
            tile; PV windows identically so masked columns are never read."""
            qmat = qkT["qs" if stream == "star" else "qh"]
            kmat = qkT["ks"]
            dpat = d_incl if stream == "star" else d_strict
            ohs = ohss[stream]
            last_kt = 4 * qt + 3
            nkt = last_kt + 1
            nround = (nkt + 1) // 2

            if True:
                hp = pair
                uref = {}
                pos = {0: psO.tile([65, 512], F32, tag="po",
                                   name=f"po{stream}{qt}{pair}0"),
                       1: psO.tile([65, 512], F32, tag="po",
                                   name=f"po{stream}{qt}{pair}1")}
                for kt in range(nkt):
                    r = kt - 4 * qt
                    w0 = r * 128 if r > 0 else 0
                    us = {}
                    for h01 in range(2):
                        hb = h01 * 64
                        acc = psS.tile([128, 512], F32, tag="s",
                                       name=f"S{stream}{qt}{pair}{h01}{kt % 2}")
                        us[h01] = (acc, pu.tile([128, 512], BF, tag="u",
                                                name=f"u{stream}{qt}{pair}{h01}{kt % 3}"))
                        nc.tensor.matmul(
                            acc[:, w0:512],
                            kmat[hb:hb + 64, hp, kt * 128:(kt + 1) * 128],
                            qmat[hb:hb + 64, hp, qt * 512 + w0:(qt + 1) * 512],
                            start=True, stop=True)
                    for h01 in range(2):
                        h = 2 * pair + h01
                        acc, u = us[h01]
                        nc.scalar.activation(u[:, w0:512], acc[:, w0:512],
                                             AF.Exp, scale=SCALE)
                        if r >= 0:          # diagonal straddle: mask strip
                            eng = nc.vector if (kt + h) % 2 == 0 else nc.gpsimd
                            eng.tensor_mul(u[:, w0:w0 + 128],
                                           u[:, w0:w0 + 128], dpat)
                            if stream == "hat":
                                E = ped.tile([128, 128], BF, tag="ed",
                                             name=f"E{qt}{pair}{h01}{r}")
                                nc.vector.tensor_scalar_mul(
                                    E, idn, eT[:, kt * HPG + h:kt * HPG + h + 1])
                                eng2 = nc.gpsimd if (kt + h) % 2 == 0 else nc.vector
                                eng2.tensor_add(u[:, w0:w0 + 128],
                                                u[:, w0:w0 + 128], E)
                                uref[(h01, kt)] = E
                    for h01 in range(2):
                        h = 2 * pair + h01
                        last = (stream == "star") and (kt == nkt - 1)
                        nc.tensor.matmul(
                            pos[h01][:, w0:512],
                            vs_aug[:, kt, h * 65:h * 65 + 65],
                            us[h01][1][:, w0:512],
                            start=(kt == 0), stop=last)
                for h01 in range(2):
                    h = 2 * pair + h01
                    if stream == "hat":
                        for r in range(4):
                            kt = 4 * qt + r
                            nc.tensor.matmul(
                                pos[h01][0:64, r * 128:(r + 1) * 128],
                                vh_raw[:, kt, h * 64:h * 64 + 64],
                                uref[(h01, kt)],
                                start=False, stop=(r == 3))
                    nc.vector.tensor_copy(ohs[:, h, qt * 512:(qt + 1) * 512],
                                          pos[h01])

        def z_norm_pair(stream, qt, pair):
            """1/Z via Ln then Exp(-x) for one head pair, then normalize
            y^T for those heads (overlaps the other pair's attention)."""
            ohs = ohss[stream]
            yT = yTs[stream]
            zl = pz.tile([1, 2 * 512], F32, tag="zl", name=f"zl{stream}{qt}{pair}")
            zr = pz.tile([1, 2 * 512], BF, tag="zr", name=f"zr{stream}{qt}{pair}")
            nc.scalar.activation(
                zl, ohs[64:65, 2 * pair:2 * pair + 2, qt * 512:(qt + 1) * 512],
                AF.Ln)
            nc.scalar.activation(zr, zl, AF.Exp, scale=-1.0)
            for h01 in range(2):
                h = 2 * pair + h01
                hb, hp = (h % 2) * 64, h // 2
                pb = psC.tile([64, 512], F32, tag="c", name=f"pb{stream}{qt}{h}")
                nc.tensor.matmul(pb, onesb[0:1, 0:64],
                                 zr[:, h01 * 512:(h01 + 1) * 512],
                                 start=True, stop=True)
                nc.vector.tensor_mul(yT[hb:hb + 64, hp, qt * 512:(qt + 1) * 512],
                                     ohs[0:64, h, qt * 512:(qt + 1) * 512], pb)

        def cproj(stream, qt, out_dram):
            """o^T = Wp^T y^T for this q-window; bias is per-partition."""
            yT = yTs[stream]
            for cb in range(8):
                pc = psC.tile([128, 512], F32, tag="c", name=f"pc{stream}{qt}{cb}")
                for p2 in range(2):
                    nc.tensor.matmul(pc, swp[:, p2, cb * 128:(cb + 1) * 128],
                                     yT[:, p2, qt * 512:(qt + 1) * 512],
                                     start=(p2 == 0), stop=(p2 == 1))
                ost = pout.tile([128, 512], BF, tag="o", name=f"ost{stream}{qt}{cb}")
                if cb % 2 == 0:
                    nc.scalar.activation(ost, pc, AF.Identity,
                                         bias=bpc[:, cb:cb + 1], scale=1.0)
                else:
                    nc.vector.tensor_scalar_add(ost, pc, bpc[:, cb:cb + 1])
                eng = nc.sync if cb % 2 == 0 else nc.gpsimd
                eng.dma_start(out_dram[cb * 128:(cb + 1) * 128,
                                       qt * 512:(qt + 1) * 512], ost)

        # ---- program order --------------------------------------------
        yTs = {"star": pyt.tile([128, 2, T], BF, tag="yt", name="yT_s"),
               "hat": pyt.tile([128, 2, T], BF, tag="yt", name="yT_h")}
        ohss = {"star": poh.tile([65, HPG, T], BF, tag="oh", name="ohs_s"),
                "hat": poh.tile([65, HPG, T], BF, tag="oh", name="ohs_h")}

        qk_star_interleaved()
        v_wave("s", vs_aug, True)
        nc.gpsimd.memset(
            vs_aug.rearrange("p k (h c) -> p k h c", c=65)[:, :, :, 64:65], 1.0)
        # x_hat + W_proj stream in while star attention runs
        for ct in range(KT):
            eng = dma_engs[ct % 3]
            eng.dma_start(sxT["h"][:, ct, :], xviews["h"][:, ct, :])
        nc.scalar.dma_start(swp, wp.rearrange("(p2 p) n -> p p2 n", p=128))

        attention_pair("star", 0, 0)
        qk_subwave("qh", "q", "h", bq, 0)
        attention_pair("star", 0, 1)
        z_norm_pair("star", 0, 0)
        qk_subwave("qh", "q", "h", bq, 1)
        z_norm_pair("star", 0, 1)
        cproj("star", 0, o_star)
        qk_subwave("kh", "k", "h", bk, 0)
        qk_subwave("kh", "k", "h", bk, 1)
        v_wave("h", vh_raw, False)
        e_rows()
        # star qt1 and hat qt0 are independent chains: interleave the pairs
        # so one stream's exp/strip phase overlaps the other's matmuls
        attention_pair("star", 1, 0)
        attention_pair("hat", 0, 0)
        z_norm_pair("star", 1, 0)
        attention_pair("star", 1, 1)
        z_norm_pair("hat", 0, 0)
        attention_pair("hat", 0, 1)
        z_norm_pair("star", 1, 1)
        cproj("star", 1, o_star)
        z_norm_pair("hat", 0, 1)
        attention_pair("hat", 1, 0)
        cproj("hat", 0, o_hat)
        z_norm_pair("hat", 1, 0)
        attention_pair("hat", 1, 1)
        z_norm_pair("hat", 1, 1)
        cproj("hat", 1, o_hat)

    nc.compile()
    return nc


def _causal_eye_masks(keep_star, keep_hat):
    tril = np.tril(np.ones((T, T), bool))
    eye = np.eye(T, dtype=bool)
    return (all(np.array_equal(keep_star[b], tril) for b in range(B))
            and all(np.array_equal(keep_hat[b], eye) for b in range(B)))


def _host_inputs(x_star, x_hat, W_attn, b_attn, W_proj, b_proj):
    """Per-core input dicts for the fast kernel."""
    import ml_dtypes
    bf = ml_dtypes.bfloat16
    f32 = np.float32
    tri = np.tril(np.ones((128, 128), f32))
    consts = dict(
        ones_in=np.ones((128, 128), bf),
        ident=np.eye(128, dtype=bf),
        diag_incl=np.ascontiguousarray(tri.T).astype(bf),      # keep k<=q
        diag_strict=np.triu(np.ones((128, 128), f32), 1).astype(bf),  # keep k<q
    )
    in_maps = []
    for core in range(G):
        b, g = divmod(core, HG)
        c0 = g * W_G
        m = dict(consts)
        m["xT_s"] = np.ascontiguousarray(x_star[b].T).astype(bf)
        m["xT_h"] = np.ascontiguousarray(x_hat[b].T).astype(bf)
        m["wq"] = np.ascontiguousarray(W_attn[:, c0:c0 + W_G]).astype(bf)
        m["wk"] = np.ascontiguousarray(W_attn[:, C + c0:C + c0 + W_G]).astype(bf)
        m["wv"] = np.ascontiguousarray(W_attn[:, 2 * C + c0:2 * C + c0 + W_G]).astype(bf)
        m["wp"] = np.ascontiguousarray(W_proj[c0:c0 + W_G, :]).astype(bf)
        m["bq_t"] = np.ascontiguousarray(
            b_attn[c0:c0 + W_G].reshape(2, 128).T.astype(f32))
        m["bk_t"] = np.ascontiguousarray(
            b_attn[C + c0:C + c0 + W_G].reshape(2, 128).T.astype(f32))
        m["bv_row"] = b_attn[2 * C + c0:2 * C + c0 + W_G].reshape(1, W_G).astype(bf)
        bp = (b_proj if g == 0 else np.zeros(C, f32))
        m["bp_cols"] = np.ascontiguousarray(bp.reshape(8, 128).T.astype(f32))
        in_maps.append(m)
    return in_maps


def _run_spmd(in_maps, **kw):
    from concourse import bass_utils
    if "fast" not in _BUILD_CACHE:
        _BUILD_CACHE["fast"] = _build_fast()
    nc = _BUILD_CACHE["fast"]
    return bass_utils.run_bass_kernel_spmd(nc, in_maps, core_ids=list(range(G)), **kw)


def _numpy_general(x_star, x_hat, keep_star, keep_hat, W_attn, b_attn,
                   W_proj, b_proj):
    """Exact reference math in numpy - fallback for non-structural masks."""
    f = np.float32

    def qkv(x):
        p = x.astype(np.float64) @ W_attn.astype(np.float64) + b_attn
        q, k, v = np.split(p, 3, axis=-1)
        r = lambda t: t.reshape(B, T, H, D).transpose(0, 2, 1, 3)
        return r(q), r(k), r(v)

    q_s, k_s, v_s = qkv(x_star)
    q_h, k_h, v_h = qkv(x_hat)
    NEG = -np.inf
    causal = np.tril(np.ones((T, T), bool))

    def soft(a):
        m = a.max(axis=-1, keepdims=True)
        m = np.where(np.isfinite(m), m, 0.0)
        e = np.exp(a - m)
        return e / e.sum(axis=-1, keepdims=True)

    def mlp(y):
        y = y.transpose(0, 2, 1, 3).reshape(B, T, C)
        return y @ W_proj.astype(np.float64) + b_proj

    att = lambda q, k: np.einsum('bhqd,bhkd->bhqk', q, k) * SCALE
    a_ss = np.where(~causal[None, None], NEG, att(q_s, k_s))
    y_star = mlp(soft(a_ss) @ v_s)
    m_s = keep_star[:, None, :, :]
    m_h = keep_hat[:, None, :, :]
    a_hs = np.where(~m_s, NEG, att(q_h, k_s))
    a_hh = np.where(~m_h, NEG, att(q_h, k_h))
    merged = np.where(np.isinf(a_hh), a_hs, a_hh)
    p = soft(merged)
    y_hat = mlp(np.where(~m_s, 0.0, p) @ v_s + np.where(~m_h, 0.0, p) @ v_h)
    return y_star.astype(f), y_hat.astype(f)


def kernel(x_star, x_hat, keep_star, keep_hat, W_attn, b_attn, W_proj, b_proj):
    x_star = np.asarray(x_star, np.float32)
    x_hat = np.asarray(x_hat, np.float32)
    keep_star = np.asarray(keep_star, bool)
    keep_hat = np.asarray(keep_hat, bool)
    W_attn = np.asarray(W_attn, np.float32)
    b_attn = np.asarray(b_attn, np.float32)
    W_proj = np.asarray(W_proj, np.float32)
    b_proj = np.asarray(b_proj, np.float32)

    if not _causal_eye_masks(keep_star, keep_hat):
        return _numpy_general(x_star, x_hat, keep_star, keep_hat,
                              W_attn, b_attn, W_proj, b_proj)

    in_maps = _host_inputs(x_star, x_hat, W_attn, b_attn, W_proj, b_proj)
    res = _run_spmd(in_maps).results

    y_star = np.zeros((B, T, C), np.float32)
    y_hat = np.zeros((B, T, C), np.float32)
    for core in range(G):
        b = core // HG
        y_star[b] += np.asarray(res[core]["o_star"]).astype(np.float32).T
        y_hat[b] += np.asarray(res[core]["o_hat"]).astype(np.float32).T
    return y_star, y_hat
